# revision 6
# baseline (speedup 1.0000x reference)
"""NT-Xent loss kernel for 8x Trainium2 NeuronCores (Bass/Tile).

Math: z = concat(z_i, z_j) [8192, 256]; zn = z / ||z||_row;
sim = (zn @ zn.T) / 0.5. Since rows are unit-norm, diag(sim) == 2.0 and all
logits lie in [-2, 2], so no max-subtraction pass is needed:
  lse_r = log(sum_j exp(sim_rj - 2) - 1) + 2      (the -1 removes the diag)
  pos_r = 2 * (zn_r . zn_partner)                 partner(r) = (r + 4096) % 8192
  loss  = mean(lse - pos)

Sharding: core k owns rows [1024k, 1024k+1024) of the sim matrix and computes
(lse - pos) for those rows against the full zn (replicated). The host sums the
8 per-core [128, 8] shards (the scalar all-reduce step) and divides by N.
"""

import os
import numpy as np

B = 4096
N = 8192
D = 256
P = 128
NCORES = 8
RPC = 1024  # rows per core
RT = RPC // P  # 8 row tiles per core
G = 8  # groups of 8 row-tiles for the full z
CHUNK = 2048  # PSUM chunk (4 banks)
HS = N // CHUNK  # 4 column chunks per row tile

_cache: dict = {}
LAST_EXEC_TIME_NS = None
LAST_RESULTS = None


def _build_nc():
    import concourse.bacc as bacc
    import concourse.tile as tile
    import concourse.mybir as mybir
    from concourse.masks import make_identity
    from contextlib import ExitStack

    f32 = mybir.dt.float32
    bf16 = mybir.dt.bfloat16
    Exp = mybir.ActivationFunctionType.Exp
    Ln = mybir.ActivationFunctionType.Ln
    MUL = mybir.AluOpType.mult
    ADD = mybir.AluOpType.add
    AxX = mybir.AxisListType.X

    nc = bacc.Bacc("TRN2", target_bir_lowering=False, debug=False, num_devices=NCORES)
    zall = nc.dram_tensor("zall", [N, D], f32, kind="ExternalInput").ap()
    zrows = nc.dram_tensor("zrows", [RPC, D], f32, kind="ExternalInput").ap()
    zpart = nc.dram_tensor("zpart", [RPC, D], f32, kind="ExternalInput").ap()
    out = nc.dram_tensor("out", [P, RT], f32, kind="ExternalOutput").ap()

    with tile.TileContext(nc) as tc, ExitStack() as ctx:
        const_pool = ctx.enter_context(tc.tile_pool(name="const", bufs=1))
        ident = const_pool.tile([P, P], bf16)
        make_identity(nc, ident[:])
        bias_m2 = const_pool.tile([P, 1], f32)
        nc.gpsimd.memset(bias_m2[:], -2.0)

        zpool = ctx.enter_context(tc.tile_pool(name="z", bufs=1))
        zg = [zpool.tile([P, 8 * D], bf16, tag=f"zg{g}", name=f"zg{g}") for g in range(G)]
        zr = zpool.tile([P, 8 * D], bf16, tag="zr")
        zp = zpool.tile([P, 8 * D], bf16, tag="zp")

        # Cast-DMA (SWDGE) f32 -> bf16; tile j of group g holds rows
        # 1024g + 128j .. +127 on partitions, D on free.
        for g in range(G):
            src = zall[1024 * g : 1024 * (g + 1), :].rearrange(
                "(n p) d -> p n d", p=P
            )
            nc.gpsimd.dma_start(
                out=zg[g][:].rearrange("p (n d) -> p n d", d=D), in_=src
            )
        nc.gpsimd.dma_start(
            out=zr[:].rearrange("p (n d) -> p n d", d=D),
            in_=zrows.rearrange("(n p) d -> p n d", p=P),
        )
        nc.gpsimd.dma_start(
            out=zp[:].rearrange("p (n d) -> p n d", d=D),
            in_=zpart.rearrange("(n p) d -> p n d", p=P),
        )

        stats = ctx.enter_context(tc.tile_pool(name="stats", bufs=1))
        ss = stats.tile([P, 80], f32)
        lns = stats.tile([P, 80], f32)
        invn = stats.tile([P, 80], f32)
        dots = stats.tile([P, RT], f32)
        posM = stats.tile([P, RT], f32)
        sums = stats.tile([P, RT * HS], f32)
        Ssum = stats.tile([P, RT], f32)
        lnS = stats.tile([P, RT], f32)
        outsb = stats.tile([P, RT], f32)

        scr_pool = ctx.enter_context(tc.tile_pool(name="scr", bufs=3))

        def norm_ss(src_ap, col0):
            # ss[:, col0:col0+8] = per-row-tile sum of squares (8 tiles/slab)
            sq = scr_pool.tile([P, 8 * D], bf16, tag="sq", name="sq")
            nc.vector.tensor_mul(sq[:], src_ap, src_ap)
            nc.vector.tensor_reduce(
                out=ss[:, col0 : col0 + 8],
                in_=sq[:].rearrange("p (n d) -> p n d", d=D),
                axis=AxX,
                op=ADD,
            )

        for g in range(G):
            norm_ss(zg[g][:], 8 * g)
        norm_ss(zr[:], 64)
        norm_ss(zp[:], 72)

        # 1/||z|| = exp(-0.5 * ln(ss)); Ln+Exp share one ACT table set.
        nc.scalar.activation(lns[:], ss[:], Ln)
        nc.scalar.activation(invn[:], lns[:], Exp, bias=0.0, scale=-0.5)

        # Normalize rows in place (bf16).
        for g in range(G):
            for j in range(8):
                n = 8 * g + j
                sl = zg[g][:, j * D : (j + 1) * D]
                nc.vector.tensor_scalar_mul(sl, sl, invn[:, n : n + 1])
        for j in range(8):
            sl = zr[:, j * D : (j + 1) * D]
            nc.vector.tensor_scalar_mul(sl, sl, invn[:, 64 + j : 65 + j])

        # pos = 2 * (zn_r . z_p) / ||z_p||   (zp stays unnormalized)
        dg = scr_pool.tile([P, 8 * D], bf16, tag="sq", name="dg")
        nc.vector.tensor_mul(dg[:], zr[:], zp[:])
        nc.vector.tensor_reduce(
            out=dots[:],
            in_=dg[:].rearrange("p (n d) -> p n d", d=D),
            axis=AxX,
            op=ADD,
        )
        for j in range(8):
            nc.vector.tensor_scalar(
                out=posM[:, j : j + 1],
                in0=dots[:, j : j + 1],
                scalar1=invn[:, 72 + j : 73 + j],
                scalar2=2.0,
                op0=MUL,
                op1=MUL,
            )

        tpool = ctx.enter_context(tc.tile_pool(name="zt", bufs=1))
        znT = [tpool.tile([P, N], bf16, tag=f"znT{k}", name=f"znT{k}") for k in range(2)]
        znrT = [tpool.tile([P, RPC], bf16, tag=f"znrT{k}", name=f"znrT{k}") for k in range(2)]

        ps_pool = ctx.enter_context(tc.tile_pool(name="ps", bufs=2, space="PSUM"))

        # Transpose own row-block -> znrT[k] [128(d-half), 1024(rows)]
        for k in range(2):
            for c2 in range(2):
                pt = ps_pool.tile([P, 512], bf16, tag="ps", name="pt")
                for b in range(4):
                    j = 4 * c2 + b
                    nc.tensor.transpose(
                        pt[:, b * P : (b + 1) * P],
                        zr[:, j * D + k * P : j * D + k * P + P],
                        ident[:],
                    )
                nc.vector.tensor_copy(
                    znrT[k][:, c2 * 512 : (c2 + 1) * 512], pt[:]
                )

        # Main loop: row tile t (128 rows) x column chunk h (2048 cols).
        # t==0 interleaves the full-z transposes just-in-time so PSUM slots
        # rotate between transpose tiles and matmul tiles without a barrier.
        for t in range(RT):
            for h in range(HS):
                if t == 0:
                    for k in range(2):
                        for q in range(4):
                            pt = ps_pool.tile([P, 512], bf16, tag="ps", name="pt")
                            for b in range(4):
                                n = 16 * h + 4 * q + b
                                g, j = divmod(n, 8)
                                nc.tensor.transpose(
                                    pt[:, b * P : (b + 1) * P],
                                    zg[g][:, j * D + k * P : j * D + k * P + P],
                                    ident[:],
                                )
                            nc.vector.tensor_copy(
                                znT[k][:, h * CHUNK + q * 512 : h * CHUNK + (q + 1) * 512],
                                pt[:],
                            )
                ps = ps_pool.tile([P, CHUNK], f32, tag="ps", name="ps")
                for k in range(2):
                    for q in range(4):
                        nc.tensor.matmul(
                            ps[:, q * 512 : (q + 1) * 512],
                            znrT[k][:, t * P : (t + 1) * P],
                            znT[k][:, h * CHUNK + q * 512 : h * CHUNK + (q + 1) * 512],
                            start=(k == 0),
                            stop=(k == 1),
                            skip_group_check=True,
                        )
                es = scr_pool.tile([P, CHUNK], bf16, tag="escr", name="es")
                idx = HS * t + h
                nc.scalar.activation(
                    es[:],
                    ps[:],
                    Exp,
                    bias=bias_m2[:],
                    scale=2.0,
                    accum_out=sums[:, idx : idx + 1],
                )

        # lse - pos = ln(S - 1) + 2 - pos
        nc.vector.tensor_reduce(
            out=Ssum[:],
            in_=sums[:].rearrange("p (t h) -> p t h", h=HS),
            axis=AxX,
            op=ADD,
        )
        nc.vector.tensor_scalar_add(Ssum[:], Ssum[:], -1.0)
        nc.scalar.activation(lnS[:], Ssum[:], Ln)
        nc.vector.tensor_sub(outsb[:], lnS[:], posM[:])
        nc.vector.tensor_scalar_add(outsb[:], outsb[:], 2.0)
        nc.sync.dma_start(out=out[:], in_=outsb[:])

    nc.compile()
    return nc


def _install_ntff_hook():
    """Provide antenv.axon_hooks (absent in this image) so trace=True can
    capture NTFF profiles via libaxon_pjrt's C ABI."""
    import sys, types, ctypes, contextlib

    try:
        from antenv.axon_hooks import get_axon_ntff_profile_hook  # noqa: F401

        return True
    except ImportError:
        pass
    try:
        import antenv

        lib = ctypes.CDLL("/opt/axon/libaxon_pjrt.so")
        if not hasattr(lib, "axon_start_nrt_profile"):
            return False
        lib.axon_start_nrt_profile.argtypes = [
            ctypes.POINTER(ctypes.c_int64),
            ctypes.c_size_t,
        ]
        lib.axon_start_nrt_profile.restype = ctypes.c_int64
        lib.axon_stop_nrt_profile.argtypes = [ctypes.c_char_p]
        lib.axon_stop_nrt_profile.restype = ctypes.c_int64

        @contextlib.contextmanager
        def _hook(output_dir, device_ids):
            import jax

            jax.devices()
            if device_ids:
                ids = (ctypes.c_int64 * len(device_ids))(*device_ids)
                rc = lib.axon_start_nrt_profile(ids, len(device_ids))
            else:
                rc = lib.axon_start_nrt_profile(None, 0)
            if rc != 0:
                raise RuntimeError(f"axon_start_nrt_profile rc={rc}")
            try:
                yield
            finally:
                n = lib.axon_stop_nrt_profile(str(output_dir).encode())
                print(f"ntff profile: {n} file(s) written to {output_dir}")

        mod = types.ModuleType("antenv.axon_hooks")
        _state = {"hook": _hook}
        mod.set_axon_ntff_profile_hook = lambda h: _state.__setitem__("hook", h)
        mod.get_axon_ntff_profile_hook = lambda: _state["hook"]
        sys.modules["antenv.axon_hooks"] = mod
        antenv.axon_hooks = mod
        return True
    except Exception as e:
        print(f"ntff hook install failed: {e}")
        return False


def _get_nc():
    if "nc" not in _cache:
        _cache["nc"] = _build_nc()
    return _cache["nc"]


def kernel(z_i, z_j):
    global LAST_EXEC_TIME_NS, LAST_RESULTS
    from concourse.bass_utils import run_bass_kernel_spmd

    z = np.ascontiguousarray(
        np.concatenate([np.asarray(z_i), np.asarray(z_j)], axis=0), dtype=np.float32
    )
    in_maps = []
    for k in range(NCORES):
        lo = RPC * k
        plo = (lo + B) % N
        in_maps.append(
            {
                "zall": z,
                "zrows": np.ascontiguousarray(z[lo : lo + RPC]),
                "zpart": np.ascontiguousarray(z[plo : plo + RPC]),
            }
        )

    nc = _get_nc()
    trace = os.environ.get("BASS_KERNEL_TRACE", "0") == "1"
    if trace:
        trace = _install_ntff_hook()
    res = run_bass_kernel_spmd(nc, in_maps, core_ids=list(range(NCORES)), trace=trace)
    LAST_RESULTS = res
    LAST_EXEC_TIME_NS = res.exec_time_ns

    total = 0.0
    for k in range(NCORES):
        total += float(np.sum(np.asarray(res.results[k]["out"], dtype=np.float64)))
    return np.array(total / N, dtype=np.float32)


# revision 7
# speedup vs baseline: 1.0417x; 1.0417x over previous
"""NT-Xent loss kernel for 8x Trainium2 NeuronCores (Bass/Tile).

Math: z = concat(z_i, z_j) [8192, 256]; zn = z / ||z||_row;
sim = (zn @ zn.T) / 0.5. Since rows are unit-norm, diag(sim) == 2.0 and all
logits lie in [-2, 2], so no max-subtraction pass is needed:
  lse_r = log(sum_j exp(sim_rj - 2) - 1) + 2      (the -1 removes the diag)
  pos_r = 2 * (zn_r . zn_partner)                 partner(r) = (r + 4096) % 8192
  loss  = mean(lse - pos)

Sharding: core k owns rows [1024k, 1024k+1024) of the sim matrix and computes
(lse - pos) for those rows against the full zn (replicated). The host sums the
8 per-core [128, 8] shards (the scalar all-reduce step) and divides by N.
"""

import os
import numpy as np

B = 4096
N = 8192
D = 256
P = 128
NCORES = 8
RPC = 1024  # rows per core
RT = RPC // P  # 8 row tiles per core
G = 8  # groups of 8 row-tiles for the full z
CHUNK = 2048  # PSUM chunk (4 banks)
HS = N // CHUNK  # 4 column chunks per row tile

_cache: dict = {}
LAST_EXEC_TIME_NS = None
LAST_RESULTS = None


def _build_nc():
    import concourse.bacc as bacc
    import concourse.tile as tile
    import concourse.mybir as mybir
    from concourse.masks import make_identity
    from contextlib import ExitStack

    f32 = mybir.dt.float32
    bf16 = mybir.dt.bfloat16
    Exp = mybir.ActivationFunctionType.Exp
    Ln = mybir.ActivationFunctionType.Ln
    MUL = mybir.AluOpType.mult
    ADD = mybir.AluOpType.add
    AxX = mybir.AxisListType.X

    nc = bacc.Bacc("TRN2", target_bir_lowering=False, debug=False, num_devices=NCORES)
    zall = nc.dram_tensor("zall", [N, D], f32, kind="ExternalInput").ap()
    zrows = nc.dram_tensor("zrows", [RPC, D], f32, kind="ExternalInput").ap()
    zpart = nc.dram_tensor("zpart", [RPC, D], f32, kind="ExternalInput").ap()
    out = nc.dram_tensor("out", [P, RT], f32, kind="ExternalOutput").ap()

    with tile.TileContext(nc) as tc, ExitStack() as ctx:
        const_pool = ctx.enter_context(tc.tile_pool(name="const", bufs=1))
        ident = const_pool.tile([P, P], bf16)
        make_identity(nc, ident[:])
        bias_m2 = const_pool.tile([P, 1], f32)
        nc.gpsimd.memset(bias_m2[:], -2.0)

        zpool = ctx.enter_context(tc.tile_pool(name="z", bufs=1))
        zg = [zpool.tile([P, 8 * D], bf16, tag=f"zg{g}", name=f"zg{g}") for g in range(G)]
        zr = zpool.tile([P, 8 * D], bf16, tag="zr")
        zp = zpool.tile([P, 8 * D], bf16, tag="zp")

        # Cast-DMA (SWDGE) f32 -> bf16. Layout: partition p of group g holds
        # rows 1024g + 8p .. 1024g + 8p + 7 (8KB contiguous DRAM read per
        # partition -> large DMA descriptors). "Row tile" j = column slice
        # [:, jD:(j+1)D] = rows {1024g + 8p + j}; the relabeling is harmless:
        # rhs covers every row exactly once and the output is summed.
        for g in range(G):
            src = zall[1024 * g : 1024 * (g + 1), :].rearrange(
                "(p n) d -> p n d", p=P
            )
            nc.gpsimd.dma_start(
                out=zg[g][:].rearrange("p (n d) -> p n d", d=D), in_=src
            )
        nc.gpsimd.dma_start(
            out=zr[:].rearrange("p (n d) -> p n d", d=D),
            in_=zrows.rearrange("(p n) d -> p n d", p=P),
        )
        nc.gpsimd.dma_start(
            out=zp[:].rearrange("p (n d) -> p n d", d=D),
            in_=zpart.rearrange("(p n) d -> p n d", p=P),
        )

        stats = ctx.enter_context(tc.tile_pool(name="stats", bufs=1))
        ss = stats.tile([P, 80], f32)
        lns = stats.tile([P, 80], f32)
        invn = stats.tile([P, 80], f32)
        dots = stats.tile([P, RT], f32)
        posM = stats.tile([P, RT], f32)
        sums = stats.tile([P, RT * HS], f32)
        Ssum = stats.tile([P, RT], f32)
        lnS = stats.tile([P, RT], f32)
        outsb = stats.tile([P, RT], f32)

        scr_pool = ctx.enter_context(tc.tile_pool(name="scr", bufs=3))

        def norm_ss(src_ap, col0):
            # ss[:, col0:col0+8] = per-row-tile sum of squares (8 tiles/slab)
            sq = scr_pool.tile([P, 8 * D], bf16, tag="sq", name="sq")
            nc.vector.tensor_mul(sq[:], src_ap, src_ap)
            nc.vector.tensor_reduce(
                out=ss[:, col0 : col0 + 8],
                in_=sq[:].rearrange("p (n d) -> p n d", d=D),
                axis=AxX,
                op=ADD,
            )

        for g in range(G):
            norm_ss(zg[g][:], 8 * g)
        norm_ss(zr[:], 64)
        norm_ss(zp[:], 72)

        # 1/||z|| = exp(-0.5 * ln(ss)); Ln+Exp share one ACT table set.
        nc.scalar.activation(lns[:], ss[:], Ln)
        nc.scalar.activation(invn[:], lns[:], Exp, bias=0.0, scale=-0.5)

        # Normalize rows in place (bf16).
        for g in range(G):
            for j in range(8):
                n = 8 * g + j
                sl = zg[g][:, j * D : (j + 1) * D]
                nc.vector.tensor_scalar_mul(sl, sl, invn[:, n : n + 1])
        for j in range(8):
            sl = zr[:, j * D : (j + 1) * D]
            nc.vector.tensor_scalar_mul(sl, sl, invn[:, 64 + j : 65 + j])

        # pos = 2 * (zn_r . z_p) / ||z_p||   (zp stays unnormalized)
        dg = scr_pool.tile([P, 8 * D], bf16, tag="sq", name="dg")
        nc.vector.tensor_mul(dg[:], zr[:], zp[:])
        nc.vector.tensor_reduce(
            out=dots[:],
            in_=dg[:].rearrange("p (n d) -> p n d", d=D),
            axis=AxX,
            op=ADD,
        )
        for j in range(8):
            nc.vector.tensor_scalar(
                out=posM[:, j : j + 1],
                in0=dots[:, j : j + 1],
                scalar1=invn[:, 72 + j : 73 + j],
                scalar2=2.0,
                op0=MUL,
                op1=MUL,
            )

        tpool = ctx.enter_context(tc.tile_pool(name="zt", bufs=1))
        znT = [tpool.tile([P, N], bf16, tag=f"znT{k}", name=f"znT{k}") for k in range(2)]
        znrT = [tpool.tile([P, RPC], bf16, tag=f"znrT{k}", name=f"znrT{k}") for k in range(2)]

        ps_pool = ctx.enter_context(tc.tile_pool(name="ps", bufs=2, space="PSUM"))

        # Transpose own row-block -> znrT[k] [128(d-half), 1024(rows)]
        for k in range(2):
            for c2 in range(2):
                pt = ps_pool.tile([P, 512], bf16, tag="ps", name="pt")
                for b in range(4):
                    j = 4 * c2 + b
                    nc.tensor.transpose(
                        pt[:, b * P : (b + 1) * P],
                        zr[:, j * D + k * P : j * D + k * P + P],
                        ident[:],
                    )
                nc.vector.tensor_copy(
                    znrT[k][:, c2 * 512 : (c2 + 1) * 512], pt[:]
                )

        # Main loop: row tile t (128 rows) x column chunk h (2048 cols).
        # t==0 interleaves the full-z transposes just-in-time so PSUM slots
        # rotate between transpose tiles and matmul tiles without a barrier.
        for t in range(RT):
            for h in range(HS):
                if t == 0:
                    for k in range(2):
                        for q in range(4):
                            pt = ps_pool.tile([P, 512], bf16, tag="ps", name="pt")
                            for b in range(4):
                                n = 16 * h + 4 * q + b
                                g, j = divmod(n, 8)
                                nc.tensor.transpose(
                                    pt[:, b * P : (b + 1) * P],
                                    zg[g][:, j * D + k * P : j * D + k * P + P],
                                    ident[:],
                                )
                            nc.vector.tensor_copy(
                                znT[k][:, h * CHUNK + q * 512 : h * CHUNK + (q + 1) * 512],
                                pt[:],
                            )
                ps = ps_pool.tile([P, CHUNK], f32, tag="ps", name="ps")
                for k in range(2):
                    for q in range(4):
                        nc.tensor.matmul(
                            ps[:, q * 512 : (q + 1) * 512],
                            znrT[k][:, t * P : (t + 1) * P],
                            znT[k][:, h * CHUNK + q * 512 : h * CHUNK + (q + 1) * 512],
                            start=(k == 0),
                            stop=(k == 1),
                            skip_group_check=True,
                        )
                es = scr_pool.tile([P, CHUNK], bf16, tag="escr", name="es")
                idx = HS * t + h
                nc.scalar.activation(
                    es[:],
                    ps[:],
                    Exp,
                    bias=bias_m2[:],
                    scale=2.0,
                    accum_out=sums[:, idx : idx + 1],
                )

        # lse - pos = ln(S - 1) + 2 - pos
        nc.vector.tensor_reduce(
            out=Ssum[:],
            in_=sums[:].rearrange("p (t h) -> p t h", h=HS),
            axis=AxX,
            op=ADD,
        )
        nc.vector.tensor_scalar_add(Ssum[:], Ssum[:], -1.0)
        nc.scalar.activation(lnS[:], Ssum[:], Ln)
        nc.vector.tensor_sub(outsb[:], lnS[:], posM[:])
        nc.vector.tensor_scalar_add(outsb[:], outsb[:], 2.0)
        nc.sync.dma_start(out=out[:], in_=outsb[:])

    nc.compile()
    return nc


def _install_ntff_hook():
    """Provide antenv.axon_hooks (absent in this image) so trace=True can
    capture NTFF profiles via libaxon_pjrt's C ABI."""
    import sys, types, ctypes, contextlib

    try:
        from antenv.axon_hooks import get_axon_ntff_profile_hook  # noqa: F401

        return True
    except ImportError:
        pass
    try:
        import antenv

        lib = ctypes.CDLL("/opt/axon/libaxon_pjrt.so")
        if not hasattr(lib, "axon_start_nrt_profile"):
            return False
        lib.axon_start_nrt_profile.argtypes = [
            ctypes.POINTER(ctypes.c_int64),
            ctypes.c_size_t,
        ]
        lib.axon_start_nrt_profile.restype = ctypes.c_int64
        lib.axon_stop_nrt_profile.argtypes = [ctypes.c_char_p]
        lib.axon_stop_nrt_profile.restype = ctypes.c_int64

        @contextlib.contextmanager
        def _hook(output_dir, device_ids):
            import jax

            jax.devices()
            if device_ids:
                ids = (ctypes.c_int64 * len(device_ids))(*device_ids)
                rc = lib.axon_start_nrt_profile(ids, len(device_ids))
            else:
                rc = lib.axon_start_nrt_profile(None, 0)
            if rc != 0:
                raise RuntimeError(f"axon_start_nrt_profile rc={rc}")
            try:
                yield
            finally:
                n = lib.axon_stop_nrt_profile(str(output_dir).encode())
                print(f"ntff profile: {n} file(s) written to {output_dir}")

        mod = types.ModuleType("antenv.axon_hooks")
        _state = {"hook": _hook}
        mod.set_axon_ntff_profile_hook = lambda h: _state.__setitem__("hook", h)
        mod.get_axon_ntff_profile_hook = lambda: _state["hook"]
        sys.modules["antenv.axon_hooks"] = mod
        antenv.axon_hooks = mod
        return True
    except Exception as e:
        print(f"ntff hook install failed: {e}")
        return False


def _get_nc():
    if "nc" not in _cache:
        _cache["nc"] = _build_nc()
    return _cache["nc"]


def kernel(z_i, z_j):
    global LAST_EXEC_TIME_NS, LAST_RESULTS
    from concourse.bass_utils import run_bass_kernel_spmd

    z = np.ascontiguousarray(
        np.concatenate([np.asarray(z_i), np.asarray(z_j)], axis=0), dtype=np.float32
    )
    in_maps = []
    for k in range(NCORES):
        lo = RPC * k
        plo = (lo + B) % N
        in_maps.append(
            {
                "zall": z,
                "zrows": np.ascontiguousarray(z[lo : lo + RPC]),
                "zpart": np.ascontiguousarray(z[plo : plo + RPC]),
            }
        )

    nc = _get_nc()
    trace = os.environ.get("BASS_KERNEL_TRACE", "0") == "1"
    if trace:
        trace = _install_ntff_hook()
    res = run_bass_kernel_spmd(nc, in_maps, core_ids=list(range(NCORES)), trace=trace)
    LAST_RESULTS = res
    LAST_EXEC_TIME_NS = res.exec_time_ns

    total = 0.0
    for k in range(NCORES):
        total += float(np.sum(np.asarray(res.results[k]["out"], dtype=np.float64)))
    return np.array(total / N, dtype=np.float32)
